# revision 6
# baseline (speedup 1.0000x reference)
"""Trainium2 Bass kernel for nn_CustomLoss_35940286333129.

loss[b] = mean|pred-target| (mae, scalar)
        + mean(min_n cdist[b,n,m]) + mean(min_b cdist[b,n,m])  (chamfer, scalar)
        + mean|sort(pred[b].ravel()) - sort(target[b].ravel())|  (emd, per-b)

Sharding: data-parallel over batch B=32 across 8 NeuronCores (4 samples each).

Per-core device kernel (per local sample b, with P=pred[b] [1024,128],
T=target[b] [1024,128]):
  - d2[m,n] = ||T[m]||^2 + ||P[n]||^2 - 2*T[m].P[n] computed tile-wise on the
    TensorEngine as two accumulating matmuls into one PSUM tile:
      (1) lhsT = -2*T^t (stationary [128,128]), rhs = P^t (moving [128,512])
      (2) K=2 rank-2 bias matmul: [tn;ones]^T @ [ones;pn] adds tn[m]+pn[n]
    P^t/T^t built on-chip via PE transposes; norms via ACT Square+accum;
    norm row vectors flattened partition->free via a DRAM bounce.
  - chamfer ingredients: free-dim min over n per (b,m) (DVE reduce) and a
    running elementwise min over local b (DVE min) kept in SBUF, sqrt'd at the
    end (sqrt commutes with min).
  - mae partial sums via DVE |P-T| reduce.
Host combines: cross-core elementwise min (+mean) for the chamfer axis=0 term,
means for the rest, and the exact per-sample EMD via np.sort on the cores'
device-untouched term (sort is unsupported on trn2; it is 0.015% of the
output value).
"""

import numpy as np

B, N, D = 32, 1024, 128
NCORES = 8
BL = B // NCORES          # 4 local samples per core
NT = N // 128             # 8 row tiles
CH = 512                  # psum chunk (free dim)
NCH = N // CH             # 2 chunks

_CACHE = {}


def _build():
    import concourse.bass as bass
    import concourse.bacc as bacc
    import concourse.tile as tile
    from concourse import mybir, masks

    f32, f32r = mybir.dt.float32, mybir.dt.float32r
    AL = mybir.AluOpType
    AF = mybir.ActivationFunctionType
    AX = mybir.AxisListType

    nc = bacc.Bacc("TRN2", target_bir_lowering=False, debug=False,
                   num_devices=NCORES)
    pred = nc.declare_dram_parameter("pred", [BL, N, D], f32, isOutput=False)
    targ = nc.declare_dram_parameter("target", [BL, N, D], f32, isOutput=False)
    mae_o = nc.declare_dram_parameter("mae_part", [128, BL], f32, isOutput=True)
    ch1_o = nc.declare_dram_parameter("ch1_part", [128, BL * NT], f32,
                                      isOutput=True)
    ch0_o = nc.declare_dram_parameter("ch0_part", [N, N], f32, isOutput=True)

    with tile.TileContext(nc) as tc:
        with (
            tc.tile_pool(name="const", bufs=1) as constp,
            tc.tile_pool(name="nat", bufs=2) as natp,
            tc.tile_pool(name="natT", bufs=2) as natTp,
            tc.tile_pool(name="mm", bufs=2) as mmp,
            tc.tile_pool(name="mmT", bufs=2) as mmTp,
            tc.tile_pool(name="bias", bufs=2) as biasp,
            tc.tile_pool(name="small", bufs=2) as smallp,
            tc.tile_pool(name="sq", bufs=2) as sqp,
            tc.tile_pool(name="persist", bufs=1) as perp,
            tc.tile_pool(name="gps", bufs=3, space=bass.MemorySpace.PSUM) as gps,
            tc.tile_pool(name="tps", bufs=2, space=bass.MemorySpace.PSUM) as tps,
            tc.tile_pool(name="dr", bufs=4, space=bass.MemorySpace.DRAM) as dr,
        ):
            ident = constp.tile([128, 128], f32)
            masks.make_identity(nc, ident[:])
            ones2 = constp.tile([2, N], f32)
            nc.vector.memset(ones2[:], 1.0)

            acc = perp.tile([128, NT * NCH, CH], f32, tag="acc")
            ch1z = perp.tile([128, BL * NT * NCH], f32, tag="ch1z")
            mae_t = perp.tile([128, BL], f32, tag="mae")

            for b in range(BL):
                natP = natp.tile([128, NT, 128], f32, tag="natP")
                nc.gpsimd.dma_start(
                    natP[:], pred[b].rearrange("(t p) d -> p t d", p=128))
                natT = natTp.tile([128, NT, 128], f32, tag="natT")
                nc.gpsimd.dma_start(
                    natT[:], targ[b].rearrange("(t p) d -> p t d", p=128))

                # mae partial: sum over (t, d) of |P - T| per partition
                diff = sqp.tile([128, NT, 128], f32, tag="diff")
                nc.vector.tensor_sub(diff[:], natP[:], natT[:])
                nc.vector.tensor_reduce(
                    out=mae_t[:, b:b + 1], in_=diff[:], axis=AX.XY, op=AL.add,
                    apply_absolute_value=True)

                # rank-2 bias operands: lhsT rows [tn, ones], rhs rows [ones, pn]
                bias_l = biasp.tile([2, N], f32r, tag="bias_l")
                bias_r = biasp.tile([2, N], f32r, tag="bias_r")
                nc.scalar.copy(bias_l[:], ones2[:])
                nc.scalar.copy(bias_r[:], ones2[:])

                Pt = mmp.tile([128, N], f32r, tag="Pt")
                Tt2 = mmTp.tile([128, N], f32r, tag="Tt2")
                for nat, dest, scale, brow in (
                    (natP, Pt, 1.0, (1, 1)),
                    (natT, Tt2, -2.0, (0, 0)),
                ):
                    ncol = smallp.tile([128, NT], f32, tag="ncol")
                    for t in range(NT):
                        sq = sqp.tile([128, 128], f32, tag="sqs")
                        nc.scalar.activation(
                            out=sq[:], in_=nat[:, t, :], func=AF.Square,
                            accum_out=ncol[:, t:t + 1])
                        tp = tps.tile([128, 128], f32, tag="tp")
                        nc.tensor.transpose(tp[:], nat[:, t, :], ident[:])
                        nc.scalar.mul(dest[:, t * 128:(t + 1) * 128], tp[:],
                                      scale)
                    # norms -> row layout via PE transpose + DRAM bounce
                    ntp = tps.tile([128, 128], f32, tag="tp")
                    nc.tensor.transpose(ntp[:NT, :], ncol[:], ident[:])
                    nrow = smallp.tile([NT, 128], f32r, tag="nrow")
                    nc.scalar.copy(nrow[:], ntp[:NT, :])
                    dscr = dr.tile([NT, 128], f32r, tag="dscr")
                    nc.gpsimd.dma_start(dscr[:], nrow[:])
                    bt = bias_r if brow[0] else bias_l
                    r0 = brow[1]
                    nc.gpsimd.dma_start(
                        bt[r0:r0 + 1, :],
                        dscr.rearrange("a b -> () (a b)"))

                for mt in range(NT):
                    for c in range(NCH):
                        g = gps.tile([128, CH], f32, tag="g")
                        nc.tensor.matmul(
                            g[:], Tt2[:, mt * 128:(mt + 1) * 128],
                            Pt[:, c * CH:(c + 1) * CH],
                            start=True, stop=False)
                        nc.tensor.matmul(
                            g[:], bias_l[:, mt * 128:(mt + 1) * 128],
                            bias_r[:, c * CH:(c + 1) * CH],
                            start=False, stop=True)
                        col = (b * NT + mt) * NCH + c
                        nc.vector.tensor_reduce(
                            out=ch1z[:, col:col + 1], in_=g[:], axis=AX.X,
                            op=AL.min)
                        pos = mt * NCH + c
                        if b == 0:
                            nc.vector.tensor_copy(acc[:, pos, :], g[:])
                        else:
                            nc.vector.tensor_tensor(
                                out=acc[:, pos, :], in0=g[:],
                                in1=acc[:, pos, :], op=AL.min)

            # chamfer1: min over the two chunks, then sqrt
            ch1m = perp.tile([128, BL * NT], f32, tag="ch1m")
            nc.vector.tensor_reduce(
                out=ch1m[:], in_=ch1z.rearrange("p (k c) -> p k c", c=NCH),
                axis=AX.X, op=AL.min)
            nc.scalar.sqrt(ch1m[:], ch1m[:])
            nc.gpsimd.dma_start(ch1_o[:], ch1m[:])
            nc.gpsimd.dma_start(mae_o[:], mae_t[:])

            # chamfer0 partial: sqrt of per-core min-over-b, then store
            for i in range(NT * NCH):
                nc.scalar.sqrt(acc[:, i, :], acc[:, i, :])
            for mt in range(NT):
                nc.gpsimd.dma_start(
                    ch0_o[mt * 128:(mt + 1) * 128, :].rearrange(
                        "p (c j) -> p c j", c=NCH),
                    acc[:, mt * NCH:(mt + 1) * NCH, :])

    nc.compile()
    return nc


def _get_nc():
    if "nc" not in _CACHE:
        _CACHE["nc"] = _build()
    return _CACHE["nc"]


def run_device(pred, target, trace=False, **kw):
    from concourse.bass_utils import run_bass_kernel_spmd

    nc = _get_nc()
    ins = []
    for i in range(NCORES):
        sl = slice(i * BL, (i + 1) * BL)
        ins.append({
            "pred": np.ascontiguousarray(pred[sl], dtype=np.float32),
            "target": np.ascontiguousarray(target[sl], dtype=np.float32),
        })
    return run_bass_kernel_spmd(nc, ins, list(range(NCORES)), trace=trace, **kw)


def kernel(pred, target):
    pred = np.asarray(pred, dtype=np.float32)
    target = np.asarray(target, dtype=np.float32)
    res = run_device(pred, target)
    rs = res.results

    mae = np.sum([r["mae_part"].astype(np.float64).sum() for r in rs])
    mae /= float(B * N * D)

    ch1 = np.mean([r["ch1_part"].astype(np.float64).mean() for r in rs])

    d0 = rs[0]["ch0_part"]
    for r in rs[1:]:
        d0 = np.minimum(d0, r["ch0_part"])
    ch0 = d0.astype(np.float64).mean()

    p = np.sort(pred.reshape(B, -1), axis=1)
    g = np.sort(target.reshape(B, -1), axis=1)
    emd = np.abs(p - g).mean(axis=1, dtype=np.float64)

    return (mae + ch0 + ch1 + emd).astype(np.float32)


# revision 8
# speedup vs baseline: 1.0925x; 1.0925x over previous
"""Trainium2 Bass kernel for nn_CustomLoss_35940286333129.

loss[b] = mean|pred-target| (mae, scalar)
        + mean(min_n cdist[b,n,m]) + mean(min_b cdist[b,n,m])  (chamfer, scalar)
        + mean|sort(pred[b].ravel()) - sort(target[b].ravel())|  (emd, per-b)

Sharding: data-parallel over batch B=32 across 8 NeuronCores (4 samples each).

Per-core device kernel (per local sample b, P=pred[b], T=target[b] [1024,128]):
  - d2[m,n] = tn[m] + pn[n] - 2*T[m].P[n], built per [128,1024] tile as:
      PSUM(fp32)  = (-2*T^t)^T @ P^t   (fp16 operands, PE)
                  + [pn_hi;pn_lo] rank-2 fp16 bias matmul  (adds pn[n])
      z16(fp16)   = ACT Relu(PSUM + tn[m] per-partition bias)  (d2 > 0 so
                    Relu is the identity; one pass converts to fp16 SBUF)
  - chamfer: DVE fp16 4x-mode reduce (min over n per (b,m)) + running
    elementwise min over local b; host finishes the cross-core min + sqrt.
  - transposes of P/T on the PE (fp16), norms via DVE square+reduce (fp32),
    pn split hi/lo in fp16 so the rank-2 bias matmul is fp32-accurate.
  - mae partial sums via DVE |P-T| reduce in fp32.
Host: means, cross-core elementwise min + sqrt for chamfer, and the exact
per-sample EMD via np.sort (sort is unsupported on trn2; EMD is 0.015% of the
output value).
"""

import numpy as np

B, N, D = 32, 1024, 128
NCORES = 8
BL = B // NCORES          # 4 local samples per core
NT = N // 128             # 8 row tiles
CH = 512                  # psum chunk (free dim)
NCH = N // CH             # 2 chunks

_CACHE = {}


def _build():
    import concourse.bass as bass
    import concourse.bacc as bacc
    import concourse.tile as tile
    from concourse import mybir, masks

    f32, f16 = mybir.dt.float32, mybir.dt.float16
    AL = mybir.AluOpType
    AF = mybir.ActivationFunctionType
    AX = mybir.AxisListType

    nc = bacc.Bacc("TRN2", target_bir_lowering=False, debug=False,
                   num_devices=NCORES)
    pred = nc.declare_dram_parameter("pred", [BL, N, D], f32, isOutput=False)
    targ = nc.declare_dram_parameter("target", [BL, N, D], f32, isOutput=False)
    mae_o = nc.declare_dram_parameter("mae_part", [128, BL], f32, isOutput=True)
    ch1_o = nc.declare_dram_parameter("ch1_part", [128, BL * NT], f32,
                                      isOutput=True)
    ch0_o = nc.declare_dram_parameter("ch0_part", [N, N], f16, isOutput=True)

    with tile.TileContext(nc) as tc:
        with (
            tc.tile_pool(name="const", bufs=1) as constp,
            tc.tile_pool(name="nat", bufs=2) as natp,
            tc.tile_pool(name="natT", bufs=2) as natTp,
            tc.tile_pool(name="nath", bufs=2) as nathp,
            tc.tile_pool(name="mm", bufs=2) as mmp,
            tc.tile_pool(name="mmT", bufs=2) as mmTp,
            tc.tile_pool(name="bias", bufs=2) as biasp,
            tc.tile_pool(name="small", bufs=3) as smallp,
            tc.tile_pool(name="sq", bufs=2) as sqp,
            tc.tile_pool(name="z", bufs=3) as zp,
            tc.tile_pool(name="persist", bufs=1) as perp,
            tc.tile_pool(name="gps", bufs=2, space=bass.MemorySpace.PSUM) as gps,
            tc.tile_pool(name="tps", bufs=2, space=bass.MemorySpace.PSUM) as tps,
            tc.tile_pool(name="nps", bufs=1, space=bass.MemorySpace.PSUM) as nps,
            tc.tile_pool(name="dr", bufs=2, space=bass.MemorySpace.DRAM) as dr,
        ):
            ident16 = constp.tile([128, 128], f16)
            masks.make_identity(nc, ident16[:])
            ident32 = constp.tile([128, 128], f32)
            masks.make_identity(nc, ident32[:])
            onesk2 = constp.tile([2, 128], f16)
            nc.vector.memset(onesk2[:], 1.0)

            acc = perp.tile([128, NT, N], f16, tag="acc")
            ch1z = perp.tile([128, BL * NT], f32, tag="ch1z")
            mae_t = perp.tile([128, BL], f32, tag="mae")

            for b in range(BL):
                natP = natp.tile([128, NT, 128], f32, tag="natP")
                nc.gpsimd.dma_start(
                    natP[:], pred[b].rearrange("(t p) d -> p t d", p=128))
                natT = natTp.tile([128, NT, 128], f32, tag="natT")
                nc.gpsimd.dma_start(
                    natT[:], targ[b].rearrange("(t p) d -> p t d", p=128))

                # mae partial: sum over (t, d) of |P - T| per partition
                diff = sqp.tile([128, NT, 128], f32, tag="diff")
                nc.vector.tensor_sub(diff[:], natP[:], natT[:])
                nc.vector.tensor_reduce(
                    out=mae_t[:, b:b + 1], in_=diff[:], axis=AX.XY, op=AL.add,
                    apply_absolute_value=True)

                # fp16 casts (T scaled by -2)
                Ph = nathp.tile([128, NT, 128], f16, tag="Ph")
                nc.vector.tensor_copy(Ph[:], natP[:])
                Th2 = nathp.tile([128, NT, 128], f16, tag="Th2")
                nc.vector.tensor_scalar_mul(Th2[:], natT[:], -2.0)

                # norms (fp32): pncol / tncol [128, NT]
                pncol = smallp.tile([128, NT], f32, tag="pncol")
                tncol = smallp.tile([128, NT], f32, tag="tncol")
                for nat, ncol in ((natP, pncol), (natT, tncol)):
                    sq = sqp.tile([128, NT, 128], f32, tag="sqs")
                    nc.vector.tensor_mul(sq[:], nat[:], nat[:])
                    nc.vector.tensor_reduce(
                        out=ncol[:], in_=sq[:], axis=AX.X, op=AL.add)

                # transposes: 8 PE fp16 transposes -> one psum bank -> SBUF
                PhT = mmp.tile([128, N], f16, tag="PhT")
                Th2T = mmTp.tile([128, N], f16, tag="Th2T")
                for nath, dest in ((Ph, PhT), (Th2, Th2T)):
                    tp = tps.tile([128, N], f16, tag="tp")
                    for t in range(NT):
                        nc.tensor.transpose(
                            tp[:, t * 128:(t + 1) * 128], nath[:, t, :],
                            ident16[:])
                    nc.vector.tensor_copy(dest[:], tp[:])

                # pn -> row layout (hi/lo fp16) via PE transpose + DRAM bounce
                ntp = nps.tile([NT, 128], f32, tag="ntp")
                nc.tensor.transpose(ntp[:], pncol[:], ident32[:])
                nrow = smallp.tile([NT, 128], f32, tag="nrow")
                nc.scalar.copy(nrow[:], ntp[:])
                hi16 = smallp.tile([NT, 128], f16, tag="hi16")
                nc.vector.tensor_copy(hi16[:], nrow[:])
                hi32 = smallp.tile([NT, 128], f32, tag="hi32")
                nc.vector.tensor_copy(hi32[:], hi16[:])
                lo16 = smallp.tile([NT, 128], f16, tag="lo16")
                nc.vector.tensor_sub(lo16[:], nrow[:], hi32[:])
                dscr = dr.tile([2, NT, 128], f16, tag="dscr")
                nc.gpsimd.dma_start(dscr[0], hi16[:])
                nc.gpsimd.dma_start(dscr[1], lo16[:])
                bias_r = biasp.tile([2, N], f16, tag="bias_r")
                nc.gpsimd.dma_start(
                    bias_r[:], dscr.rearrange("r a b -> r (a b)"))

                for mt in range(NT):
                    g = gps.tile([128, NCH, CH], f32, tag="g")
                    for c in range(NCH):
                        nc.tensor.matmul(
                            g[:, c, :], Th2T[:, mt * 128:(mt + 1) * 128],
                            PhT[:, c * CH:(c + 1) * CH],
                            start=True, stop=False)
                        nc.tensor.matmul(
                            g[:, c, :], onesk2[:],
                            bias_r[:, c * CH:(c + 1) * CH],
                            start=False, stop=True)
                    # z16 = d2 = PSUM + tn[m]  (Relu == identity, d2 > 0)
                    z16 = zp.tile([128, NCH, CH], f16, tag="z16")
                    nc.scalar.activation(
                        out=z16[:], in_=g[:], func=AF.Relu,
                        bias=tncol[:, mt:mt + 1], scale=1.0)
                    col = b * NT + mt
                    nc.vector.tensor_reduce(
                        out=ch1z[:, col:col + 1], in_=z16[:], axis=AX.XY,
                        op=AL.min)
                    if b == 0:
                        nc.vector.tensor_copy(
                            acc[:, mt, :], z16.rearrange("p c j -> p (c j)"))
                    else:
                        nc.vector.tensor_tensor(
                            out=acc[:, mt, :],
                            in0=z16.rearrange("p c j -> p (c j)"),
                            in1=acc[:, mt, :], op=AL.min)

            # chamfer1: sqrt of per-(b,m) min d2
            nc.scalar.sqrt(ch1z[:], ch1z[:])
            nc.gpsimd.dma_start(ch1_o[:], ch1z[:])
            nc.gpsimd.dma_start(mae_o[:], mae_t[:])

            # chamfer0 partial: per-core min-over-b of d2 (fp16; host sqrts)
            for mt in range(NT):
                nc.gpsimd.dma_start(
                    ch0_o[mt * 128:(mt + 1) * 128, :], acc[:, mt, :])

    nc.compile()
    return nc


def _get_nc():
    if "nc" not in _CACHE:
        _CACHE["nc"] = _build()
    return _CACHE["nc"]


def run_device(pred, target, trace=False, **kw):
    from concourse.bass_utils import run_bass_kernel_spmd

    nc = _get_nc()
    ins = []
    for i in range(NCORES):
        sl = slice(i * BL, (i + 1) * BL)
        ins.append({
            "pred": np.ascontiguousarray(pred[sl], dtype=np.float32),
            "target": np.ascontiguousarray(target[sl], dtype=np.float32),
        })
    return run_bass_kernel_spmd(nc, ins, list(range(NCORES)), trace=trace, **kw)


def kernel(pred, target):
    pred = np.asarray(pred, dtype=np.float32)
    target = np.asarray(target, dtype=np.float32)
    res = run_device(pred, target)
    rs = res.results

    mae = np.sum([r["mae_part"].astype(np.float64).sum() for r in rs])
    mae /= float(B * N * D)

    ch1 = np.mean([r["ch1_part"].astype(np.float64).mean() for r in rs])

    d0 = rs[0]["ch0_part"].astype(np.float32)
    for r in rs[1:]:
        d0 = np.minimum(d0, r["ch0_part"].astype(np.float32))
    ch0 = np.sqrt(d0.astype(np.float64)).mean()

    p = np.sort(pred.reshape(B, -1), axis=1)
    g = np.sort(target.reshape(B, -1), axis=1)
    emd = np.abs(p - g).mean(axis=1, dtype=np.float64)

    return (mae + ch0 + ch1 + emd).astype(np.float32)


# revision 10
# speedup vs baseline: 1.1861x; 1.0857x over previous
"""Trainium2 Bass kernel for nn_CustomLoss_35940286333129.

loss[b] = mean|pred-target| (mae, scalar)
        + mean(min_n cdist[b,n,m]) + mean(min_b cdist[b,n,m])  (chamfer, scalar)
        + mean|sort(pred[b].ravel()) - sort(target[b].ravel())|  (emd, per-b)

Sharding: data-parallel over batch B=32 across 8 NeuronCores (4 samples each).

Per-core device kernel (per local sample b, P=pred[b], T=target[b] [1024,128]):
  - PSUM[m, n] = -2*T[m].P[n] + pn[n]  via two accumulating fp16 matmuls
    (stationary -2*T^t tile; rank-2 [ones;ones]^T@[pn_hi;pn_lo] bias).
    Operand transposes run on the DMA crossbar (dma_start_transpose, fp16).
  - One fused custom DVE op per [128,1025] tile consumes the PSUM:
        z    = psum + tn[m]          (per-partition scalar)
        out  = where(z < 1e30, min(z, acc), running_min(z))  -> acc (fp16)
    The PSUM pad column 1024 is pre-set to 3e38, so column 1024 of `out`
    captures min_n d2 (the chamfer axis=1 ingredient) while columns 0..1023
    update the running min over local b (the chamfer axis=0 ingredient).
  - norms via ACT Square+accumulate (fp32); mae via GpSimd sub + ACT |.|-acc.
Host: means, cross-core elementwise min + sqrt for chamfer, and the exact
per-sample EMD via np.sort (sort is unsupported on trn2; EMD is 0.015% of
the output value).
"""

import numpy as np

B, N, D = 32, 1024, 128
NCORES = 8
BL = B // NCORES          # 4 local samples per core
NT = N // 128             # 8 row tiles
NPAD = N + 1              # g tile free size (1 scratch col for the scan)

_CACHE = {}


def _register_op():
    from concourse import dve_ops
    from concourse.dve_ops import DveOp, OPS, DveOpSpec
    from concourse.dve_spec import (Spec, Src0, Src1, C0, C1, C2, scan, minn,
                                    select, lower, AluOp)

    for op in OPS:
        if op.name == "MINACC_CH":
            return op

    z = Src0 + C0
    r = scan(AluOp.MIN, z, init=C2)
    body = select(z < C1, minn(z, Src1), r)

    def ref(in0, in1, s0, s1, imm2):
        zz = in0 + s0
        rr = np.minimum.accumulate(np.minimum(zz, imm2), axis=-1)
        return np.where(zz < s1, np.minimum(zz, in1), rr)

    spec = Spec(body=body, reference=ref)
    shas = {}
    for ver in ("v3", "v4"):
        tmp = DveOpSpec(name="MINACC_CH", opcode=0, uops=lower(spec, ver=ver),
                        rd1_en=True)
        shas[ver] = tmp.sha(ver)
    op = DveOp("MINACC_CH", spec, subdim=False, uops_sha=shas)
    OPS.append(op)
    dve_ops.CUSTOM_DVE_SPECS[op.name] = op.spec
    dve_ops._SUB_OPCODE_FOR_NAME[op.name] = (
        dve_ops._CUSTOM_DVE_ROW_BASE + len(OPS) - 1)
    return op


def _build():
    import concourse.bass as bass
    import concourse.bacc as bacc
    import concourse.tile as tile
    from concourse import mybir, masks

    MINACC = _register_op()

    f32, f16 = mybir.dt.float32, mybir.dt.float16
    AL = mybir.AluOpType
    AF = mybir.ActivationFunctionType

    nc = bacc.Bacc("TRN2", target_bir_lowering=False, debug=False,
                   num_devices=NCORES)
    pred = nc.declare_dram_parameter("pred", [BL, N, D], f32, isOutput=False)
    targ = nc.declare_dram_parameter("target", [BL, N, D], f32, isOutput=False)
    mae_o = nc.declare_dram_parameter("mae_part", [128, BL], f32, isOutput=True)
    ch1_o = nc.declare_dram_parameter("ch1_part", [128, BL * NT], f32,
                                      isOutput=True)
    ch0_o = nc.declare_dram_parameter("ch0_part", [N, N], f16, isOutput=True)

    with tile.TileContext(nc) as tc:
        with (
            tc.tile_pool(name="const", bufs=1) as constp,
            tc.tile_pool(name="nat", bufs=2) as natp,
            tc.tile_pool(name="natT", bufs=2) as natTp,
            tc.tile_pool(name="nath", bufs=2) as nathp,
            tc.tile_pool(name="mm", bufs=2) as mmp,
            tc.tile_pool(name="mmT", bufs=2) as mmTp,
            tc.tile_pool(name="bias", bufs=2) as biasp,
            tc.tile_pool(name="small", bufs=3) as smallp,
            tc.tile_pool(name="sq", bufs=2) as sqp,
            tc.tile_pool(name="persist", bufs=1) as perp,
            tc.tile_pool(name="gps", bufs=1, space=bass.MemorySpace.PSUM) as gps,
            tc.tile_pool(name="nps", bufs=2, space=bass.MemorySpace.PSUM) as nps,
            tc.tile_pool(name="dr", bufs=2, space=bass.MemorySpace.DRAM) as dr,
            tc.tile_pool(name="drt", bufs=2, space=bass.MemorySpace.DRAM) as drt,
        ):
            ident32 = constp.tile([128, 128], f32)
            masks.make_identity(nc, ident32[:])
            onesk2 = constp.tile([2, 128], f16)
            nc.vector.memset(onesk2[:], 1.0)

            acc = perp.tile([128, NT, NPAD], f16, tag="acc")
            nc.vector.memset(acc[:], 60000.0)
            ch1z = perp.tile([128, BL * NT], f32, tag="ch1z")
            mae_t = perp.tile([128, BL], f32, tag="mae")

            gtiles = [gps.tile([128, NPAD], f32, tag=f"g{i}", name=f"g{i}")
                      for i in range(2)]
            for gt in gtiles:
                nc.vector.memset(gt[:, N:NPAD], 3.0e38)

            for b in range(BL):
                natP = natp.tile([128, NT, 128], f32, tag="natP")
                nc.sync.dma_start(
                    natP[:], pred[b].rearrange("(t p) d -> p t d", p=128))
                natT = natTp.tile([128, NT, 128], f32, tag="natT")
                nc.sync.dma_start(
                    natT[:], targ[b].rearrange("(t p) d -> p t d", p=128))

                # mae partial: |P - T| summed per partition (GpSimd + ACT)
                diff = sqp.tile([128, NT, 128], f32, tag="diff")
                nc.gpsimd.tensor_sub(diff[:], natP[:], natT[:])
                absx = sqp.tile([128, NT * 128], f32, tag="absx")
                nc.scalar.activation(
                    out=absx[:], in_=diff.rearrange("p t d -> p (t d)"),
                    func=AF.Abs, accum_out=mae_t[:, b:b + 1])

                # norms: ACT Square + accumulate per 128-col tile (fp32)
                pncol = smallp.tile([128, NT], f32, tag="pncol")
                tncol = smallp.tile([128, NT], f32, tag="tncol")
                sq = sqp.tile([128, NT, 128], f32, tag="sqs")
                for nat, ncol in ((natP, pncol), (natT, tncol)):
                    for t in range(NT):
                        nc.scalar.activation(
                            out=sq[:, t, :], in_=nat[:, t, :], func=AF.Square,
                            accum_out=ncol[:, t:t + 1])

                # fp16 casts (T scaled by -2) then DMA-crossbar transposes
                PhT = mmp.tile([128, N], f16, tag="PhT")
                Th2T = mmTp.tile([128, N], f16, tag="Th2T")
                for nat, dest, scale in ((natP, PhT, 1.0), (natT, Th2T, -2.0)):
                    nath = nathp.tile([128, NT, 128], f16, tag="nath")
                    nc.scalar.mul(nath[:], nat[:], scale)
                    dtr = drt.tile([N, 128], f16, tag="dtr")
                    nc.sync.dma_start(
                        dtr.rearrange("(t p) d -> p t d", p=128), nath[:])
                    nc.sync.dma_start_transpose(dest[:], dtr[:])

                # pn -> fp16 hi/lo rows via PE transpose + DRAM bounce
                ntp = nps.tile([NT, 128], f32, tag="ntp")
                nc.tensor.transpose(ntp[:], pncol[:], ident32[:])
                nrow = smallp.tile([NT, 128], f32, tag="nrow")
                nc.scalar.copy(nrow[:], ntp[:])
                hi16 = smallp.tile([NT, 128], f16, tag="hi16")
                nc.vector.tensor_copy(hi16[:], nrow[:])
                hi32 = smallp.tile([NT, 128], f32, tag="hi32")
                nc.vector.tensor_copy(hi32[:], hi16[:])
                lo16 = smallp.tile([NT, 128], f16, tag="lo16")
                nc.vector.tensor_sub(lo16[:], nrow[:], hi32[:])
                dscr = dr.tile([2, NT, 128], f16, tag="dscr")
                nc.sync.dma_start(dscr[0], hi16[:])
                nc.sync.dma_start(dscr[1], lo16[:])
                bias_r = biasp.tile([2, N], f16, tag="bias_r")
                nc.sync.dma_start(bias_r[:], dscr.rearrange("r a b -> r (a b)"))

                for mt in range(NT):
                    g = gtiles[mt % 2]
                    for c in range(2):
                        nc.tensor.matmul(
                            g[:, c * 512:(c + 1) * 512],
                            Th2T[:, mt * 128:(mt + 1) * 128],
                            PhT[:, c * 512:(c + 1) * 512],
                            start=True, stop=False)
                        nc.tensor.matmul(
                            g[:, c * 512:(c + 1) * 512], onesk2[:],
                            bias_r[:, c * 512:(c + 1) * 512],
                            start=False, stop=True)
                    nc.vector._custom_dve(
                        MINACC, out=acc[:, mt, :], in0=g[:],
                        in1=acc[:, mt, :], s0=tncol[:, mt:mt + 1],
                        s1=1.0e30, imm2=3.0e38)
                # harvest this b's min_n d2 (scratch col) before the next b
                nc.vector.tensor_copy(
                    ch1z[:, b * NT:(b + 1) * NT], acc[:, :, N])

            nc.scalar.sqrt(ch1z[:], ch1z[:])
            nc.sync.dma_start(ch1_o[:], ch1z[:])
            nc.sync.dma_start(mae_o[:], mae_t[:])
            for mt in range(NT):
                nc.sync.dma_start(
                    ch0_o[mt * 128:(mt + 1) * 128, :], acc[:, mt, 0:N])

    nc.compile()
    return nc


def _get_nc():
    if "nc" not in _CACHE:
        _CACHE["nc"] = _build()
    return _CACHE["nc"]


def run_device(pred, target, trace=False, **kw):
    from concourse.bass_utils import run_bass_kernel_spmd

    nc = _get_nc()
    ins = []
    for i in range(NCORES):
        sl = slice(i * BL, (i + 1) * BL)
        ins.append({
            "pred": np.ascontiguousarray(pred[sl], dtype=np.float32),
            "target": np.ascontiguousarray(target[sl], dtype=np.float32),
        })
    return run_bass_kernel_spmd(nc, ins, list(range(NCORES)), trace=trace, **kw)


def kernel(pred, target):
    pred = np.asarray(pred, dtype=np.float32)
    target = np.asarray(target, dtype=np.float32)
    res = run_device(pred, target)
    rs = res.results

    mae = np.sum([r["mae_part"].astype(np.float64).sum() for r in rs])
    mae /= float(B * N * D)

    ch1 = np.mean([r["ch1_part"].astype(np.float64).mean() for r in rs])

    d0 = rs[0]["ch0_part"].astype(np.float32)
    for r in rs[1:]:
        d0 = np.minimum(d0, r["ch0_part"].astype(np.float32))
    ch0 = np.sqrt(d0.astype(np.float64)).mean()

    p = np.sort(pred.reshape(B, -1), axis=1)
    g = np.sort(target.reshape(B, -1), axis=1)
    emd = np.abs(p - g).mean(axis=1, dtype=np.float64)

    return (mae + ch0 + ch1 + emd).astype(np.float32)
